# revision 1
# baseline (speedup 1.0000x reference)
"""Trainium2 Bass kernel for nn_FDAF (concat -> depthwise5x5 -> InstanceNorm ->
GELU -> 1x1 conv -> bilinear warp -> subtract), data-parallel over 8 cores.

Sharding: core c = (sample n = c//2, row-half s = c%2). Each core computes both
feature outputs for its 128-row half of its sample. InstanceNorm stats are
combined across the row-half pair with a tiny AllReduce collective.

Self-contained: hardcodes N=4, C=64, H=W=256.
"""
import numpy as np
import ml_dtypes

import concourse.bass as bass
import concourse.bacc as bacc
import concourse.tile as tile
from concourse import mybir
from concourse.bass_utils import run_bass_kernel_spmd

FP32 = mybir.dt.float32
BF16 = mybir.dt.bfloat16
AL = mybir.AluOpType
AF = mybir.ActivationFunctionType
AX = mybir.AxisListType

N, C, H, W = 4, 64, 256, 256
HH = 128          # rows per core (half image)
RT = 8            # rows per tile
NT = HH // RT     # 16 tiles
FT = RT * W       # free elems per tile = 2048
FH = HH * W       # free elems per half = 32768
DSCALE = (W - 1) / (2.0 * W)  # flow -> pixel displacement (align_corners=True)

# tap split for the depthwise conv (25 taps, k = dy*5+dx).
# DVE gets even-dx taps only: odd-dx reads are 2-byte-misaligned in the bf16
# slab and would fall out of the DVE 4x/2x packing modes on real hardware;
# PE/ACT/GPSIMD reads are alignment-insensitive.
_EVEN = [k for k in range(25) if (k % 5) % 2 == 0]   # 15 taps
_ODD = [k for k in range(25) if (k % 5) % 2 == 1]    # 10 taps
DVE_TAPS = _EVEN + _ODD[:1]        # 16 taps (8 pairs; one odd tap tolerated)
PE_TAPS = _ODD[1:4]                # 3 diag-matmul direct taps
ACT_TAPS = _ODD[4:8]               # 4 scale-copy products (2 pairs)
GPS_TAPS = _ODD[8:10]              # 2 ts products (1 pair, DVE-added)

_CACHE = {}


def _build(timing=False):
    nc = bacc.Bacc("TRN2", target_bir_lowering=False, debug=False,
                   num_devices=1 if timing else 8)

    xh = nc.dram_tensor("xh", [128, 132, 260], BF16, kind="ExternalInput")
    wb = nc.dram_tensor("wb", [128, 26], FP32, kind="ExternalInput")
    pw = nc.dram_tensor("pw", [128, 4], BF16, kind="ExternalInput")
    dg = nc.dram_tensor("dg", [len(PE_TAPS) * 128, 128], BF16, kind="ExternalInput")
    ident = nc.dram_tensor("ident", [128, 128], BF16, kind="ExternalInput")
    nsw = nc.dram_tensor("nsw", [128, 128], BF16, kind="ExternalInput")
    out_d = nc.dram_tensor("out", [128, FH], FP32, kind="ExternalOutput")

    cc_in = nc.dram_tensor("cc_in", [128, 2], FP32, kind="Internal")
    cc_out = nc.dram_tensor("cc_out", [128, 2], FP32, kind="Internal")
    FH2 = FH // 2
    flow_hs = [nc.dram_tensor(f"flow_d{h}", [4, FH2], BF16, kind="Internal")
               for h in range(2)]
    WL = 64 * 260 + 8   # padded-row weight map: 260-stride rows + slack
    w9_hs = [nc.dram_tensor(f"w9_d{h}", [18, WL], BF16, kind="Internal")
             for h in range(2)]

    with tile.TileContext(nc) as tc:
        with tc.tile_pool(name="singles", bufs=1) as singles:
            xh_sb = singles.tile([128, 132, 260], BF16)
            for ci in range(11):
                r0c, r1c = ci * 12, min(132, ci * 12 + 12)
                eng = (nc.sync, nc.scalar)[ci % 2]
                eng.dma_start(out=xh_sb[:, r0c:r1c, :],
                              in_=xh.ap()[:, r0c:r1c, :])
            wb_sb = singles.tile([128, 26], FP32)
            nc.sync.dma_start(out=wb_sb, in_=wb.ap())
            pw_sb = singles.tile([128, 4], BF16)
            nc.sync.dma_start(out=pw_sb, in_=pw.ap())
            id_sb = singles.tile([128, 128], BF16)
            nc.sync.dma_start(out=id_sb, in_=ident.ap())
            nsw_sb = singles.tile([128, 128], BF16)
            nc.sync.dma_start(out=nsw_sb, in_=nsw.ap())
            dg_sb = singles.tile([128, len(PE_TAPS), 128], BF16)
            nc.sync.dma_start(out=dg_sb,
                              in_=dg.ap().rearrange("(k p) m -> p k m", p=128))
            eps_t = singles.tile([128, 1], FP32)
            nc.vector.memset(eps_t, 1e-5)
            sp = singles.tile([128, NT], FP32)
            s2p = singles.tile([128, NT], FP32)
            stat = singles.tile([128, 8], FP32)

            # ---------------- Phase A: conv + stats; Y stored bf16 -------------
            with tc.tile_pool(name="ab", bufs=1) as ab:
              y_bf = ab.tile([128, FH], BF16)
              with tc.tile_pool(name="pha", bufs=2) as pha, \
                   tc.tile_pool(name="psA", bufs=2, space="PSUM") as psA:
                for t in range(NT):
                    r0 = t * RT

                    def sl(dy, dx, r0=r0):
                        return xh_sb[:, r0 + dy:r0 + RT + dy, dx:dx + W]

                    # DVE tap pairs: ts product x2 (4x mode) + TT pair-add,
                    # each pair merged into psum by one PE identity pass
                    ps = psA.tile([128, FT], FP32)
                    nch = FT // 512
                    for ki, k in enumerate(PE_TAPS):
                        dy, dx = divmod(k, 5)
                        for j in range(nch):
                            nc.tensor.matmul(
                                ps[:, j * 512:(j + 1) * 512], dg_sb[:, ki, :],
                                xh_sb[:, r0 + dy + 2 * j:r0 + dy + 2 * j + 2,
                                      dx:dx + W],
                                start=(ki == 0), stop=False)

                    def pe_merge(src3d, last=False):
                        for j in range(nch):
                            nc.tensor.matmul(
                                ps[:, j * 512:(j + 1) * 512], id_sb,
                                src3d[:, 2 * j:2 * j + 2, :],
                                start=False, stop=(last and j == nch - 1))

                    npair = len(DVE_TAPS) // 2
                    pairs = []
                    for pi in range(npair):
                        ka, kb = DVE_TAPS[2 * pi], DVE_TAPS[2 * pi + 1]
                        pa = pha.tile([128, RT, W], BF16, tag=f"pa{pi % 2}",
                                      bufs=3)
                        pb = pha.tile([128, RT, W], BF16, tag="pb", bufs=3)
                        # fold the conv bias into the first product
                        if pi == 0:
                            nc.vector.tensor_scalar(
                                out=pa, in0=sl(*divmod(ka, 5)),
                                scalar1=wb_sb[:, ka:ka + 1],
                                scalar2=wb_sb[:, 25:26],
                                op0=AL.mult, op1=AL.add)
                        else:
                            nc.vector.tensor_scalar(
                                out=pa, in0=sl(*divmod(ka, 5)),
                                scalar1=wb_sb[:, ka:ka + 1], scalar2=None,
                                op0=AL.mult)
                        nc.vector.tensor_scalar(
                            out=pb, in0=sl(*divmod(kb, 5)),
                            scalar1=wb_sb[:, kb:kb + 1], scalar2=None, op0=AL.mult)
                        if pi < npair - 4:
                            nc.vector.tensor_tensor(out=pa, in0=pa, in1=pb,
                                                    op=AL.add)
                            pairs.append(pa)
                        else:
                            pairs.append(pa)
                            pairs.append(pb)
                    for m in pairs:
                        pe_merge(m)

                    # ACT products (scale-copy), pair-added on GPSIMD
                    aps = []
                    for ai, k in enumerate(ACT_TAPS):
                        at = pha.tile([128, RT, W], BF16, tag=f"at{ai % 2}")
                        nc.scalar.mul(out=at, in_=sl(*divmod(k, 5)),
                                      mul=wb_sb[:, k:k + 1])
                        aps.append(at)
                    nc.gpsimd.tensor_tensor(out=aps[0], in0=aps[0], in1=aps[1],
                                            op=AL.add)
                    nc.gpsimd.tensor_tensor(out=aps[2], in0=aps[2], in1=aps[3],
                                            op=AL.add)
                    pe_merge(aps[0])
                    pe_merge(aps[2])

                    # GPSIMD ts pair, DVE-added
                    ka, kb = GPS_TAPS
                    ga = pha.tile([128, RT, W], BF16, tag="ga")
                    gb = pha.tile([128, RT, W], BF16, tag="gb")
                    nc.gpsimd.tensor_scalar(out=ga, in0=sl(*divmod(ka, 5)),
                                            scalar1=wb_sb[:, ka:ka + 1],
                                            scalar2=None, op0=AL.mult)
                    nc.gpsimd.tensor_scalar(out=gb, in0=sl(*divmod(kb, 5)),
                                            scalar1=wb_sb[:, kb:kb + 1],
                                            scalar2=None, op0=AL.mult)
                    nc.vector.tensor_tensor(out=ga, in0=ga, in1=gb, op=AL.add)
                    pe_merge(ga, last=True)

                    # stats from psum + store bf16 (Copy pass writes Y;
                    # Square scratch writes the same region first, WAW-ordered)
                    nc.scalar.activation(out=y_bf[:, t * FT:(t + 1) * FT], in_=ps,
                                         func=AF.Square,
                                         accum_out=s2p[:, t:t + 1])
                    nc.scalar.activation(out=y_bf[:, t * FT:(t + 1) * FT], in_=ps,
                                         func=AF.Copy, accum_out=sp[:, t:t + 1])

              # ------------- stats combine (collective over the pair) -----------
              nc.vector.tensor_reduce(out=stat[:, 0:1], in_=sp, axis=AX.X, op=AL.add)
              nc.vector.tensor_reduce(out=stat[:, 1:2], in_=s2p, axis=AX.X, op=AL.add)
              nc.sync.dma_start(out=cc_in.ap(), in_=stat[:, 0:2])
              if not timing:
                  nc.gpsimd.collective_compute(
                      kind="AllReduce", op=AL.add,
                      replica_groups=[[0, 1], [2, 3], [4, 5], [6, 7]],
                      ins=[cc_in.ap()], outs=[cc_out.ap()])
              else:
                  nc.sync.dma_start(out=cc_out.ap(), in_=cc_in.ap())
              nc.sync.dma_start(out=stat[:, 0:2], in_=cc_out.ap())
              inv = 1.0 / (H * W)
              nc.vector.tensor_scalar(out=stat[:, 2:3], in0=stat[:, 0:1],
                                      scalar1=inv, scalar2=None, op0=AL.mult)
              nc.vector.tensor_scalar(out=stat[:, 3:4], in0=stat[:, 1:2],
                                      scalar1=inv, scalar2=None, op0=AL.mult)
              nc.vector.scalar_tensor_tensor(
                  out=stat[:, 4:5], in0=stat[:, 2:3], scalar=stat[:, 2:3],
                  in1=stat[:, 3:4], op0=AL.mult, op1=AL.subtract)
              nc.scalar.activation(out=stat[:, 5:6], in_=stat[:, 4:5],
                                   func=AF.Sqrt, scale=-1.0, bias=eps_t[:, 0:1])
              nc.vector.reciprocal(out=stat[:, 6:7], in_=stat[:, 5:6])
              nc.vector.tensor_scalar(out=stat[:, 7:8], in0=stat[:, 2:3],
                                      scalar1=stat[:, 6:7], scalar2=-1.0,
                                      op0=AL.mult, op1=AL.mult)

              # --------------- Phase B: gelu + 1x1 conv -> flow_d ---------------
              with tc.tile_pool(name="phb", bufs=2) as phb, \
                   tc.tile_pool(name="psB", bufs=2, space="PSUM") as psB:
                  for t in range(NT):
                      h, tl = t // (NT // 2), t % (NT // 2)
                      g = phb.tile([128, FT], BF16)
                      nc.scalar.activation(out=g, in_=y_bf[:, t * FT:(t + 1) * FT],
                                           func=AF.Gelu, scale=stat[:, 6:7],
                                           bias=stat[:, 7:8])
                      psf = psB.tile([4, FT], FP32)
                      for j in range(FT // 512):
                          nc.tensor.matmul(psf[:, j * 512:(j + 1) * 512], pw_sb,
                                           g[:, j * 512:(j + 1) * 512],
                                           start=True, stop=True)
                      fls = phb.tile([4, FT], BF16)
                      nc.scalar.copy(out=fls, in_=psf)
                      nc.sync.dma_start(
                          out=flow_hs[h].ap()[:, tl * FT:(tl + 1) * FT], in_=fls)

            # -------- compact weight maps: deltas -> 18 tap products ----------
            # per half: cx/cy [64, FH2/32]; part 0-31 field1, 32-63 field2
            with tc.tile_pool(name="cw", bufs=2) as cw:
              for h in range(2):
                FC = FH2 // 32
                cx = cw.tile([64, FC], BF16, tag="cx")
                cy = cw.tile([64, FC], BF16, tag="cy")
                for (dst, r1, r2) in ((cx, 0, 2), (cy, 1, 3)):
                    for (p0, row) in ((0, r1), (32, r2)):
                        nc.sync.dma_start(
                            out=dst[p0:p0 + 32, :],
                            in_=flow_hs[h].ap()[row:row + 1, :].rearrange(
                                "a (p f) -> (a p) f", p=32))
                wsel = {}
                for ax, d in (("x", cx), ("y", cy)):
                    wp = cw.tile([64, FC], BF16, tag=f"wp{ax}")
                    wm = cw.tile([64, FC], BF16, tag=f"wm{ax}")
                    w0 = cw.tile([64, FC], BF16, tag=f"w0{ax}")
                    nc.vector.tensor_scalar(out=wp, in0=d, scalar1=0.0,
                                            scalar2=None, op0=AL.max)
                    nc.vector.tensor_scalar(out=wm, in0=d, scalar1=-1.0,
                                            scalar2=0.0, op0=AL.mult, op1=AL.max)
                    nc.scalar.activation(out=w0, in_=d, func=AF.Abs)
                    nc.vector.tensor_scalar(out=w0, in0=w0, scalar1=-1.0,
                                            scalar2=1.0, op0=AL.mult, op1=AL.add)
                    wsel[ax] = {-1: wm, 0: w0, 1: wp}
                for ki, (sy, sx) in enumerate(
                        (sy, sx) for sy in (-1, 0, 1) for sx in (-1, 0, 1)):
                    p9 = cw.tile([64, FC], BF16, tag="p9")
                    nc.vector.tensor_tensor(out=p9, in0=wsel["y"][sy],
                                            in1=wsel["x"][sx], op=AL.mult)
                    wd = w9_hs[h].ap()
                    for f in range(2):
                        dst = bass.AP(tensor=wd.tensor,
                                      offset=(ki + 9 * f) * WL + 6,
                                      ap=[[520, 32], [260, 2], [1, 256]])
                        nc.sync.dma_start(out=dst, in_=p9[32 * f:32 * f + 32, :])

            # ---------------- Phase C: warp + subtract ----------------
            with tc.tile_pool(name="phc", bufs=2) as phc, \
                 tc.tile_pool(name="psC", bufs=2, space="PSUM") as psC:
                taps = [(sy, sx) for sy in (-1, 0, 1) for sx in (-1, 0, 1)]
                for t in range(NT):
                    h, tl = t // (NT // 2), t % (NT // 2)
                    w9 = w9_hs[h].ap()
                    r0 = t * RT
                    acc = psC.tile([128, FT], FP32)
                    nch = FT // 512

                    RW = RT * 260

                    def wtile(ki, sx, w9=w9, tl=tl):
                        # pre-shifted by -sx: wt[., r, c] = w9[y=tl*8+r, c-2-sx]
                        wt = phc.tile([128, RT, 260], BF16, tag=f"w9t{ki % 4}",
                                      bufs=3)
                        src = bass.AP(tensor=w9.tensor,
                                      offset=ki * WL + 4 - sx + tl * RW,
                                      ap=[[9 * WL, 2], [0, 64], [1, RW]])
                        eng = {0: nc.sync, 3: nc.sync, 1: nc.scalar,
                               4: nc.scalar}.get(ki, nc.gpsimd)
                        eng.dma_start(out=wt, in_=src)
                        return wt

                    # 9 mults on DVE over full aligned 260-wide rows; the
                    # pixel shift happens in the PE rhs read (alignment-free).
                    # Each 512-col PSUM bank needs its own start=True on tap 0.
                    for ki in range(9):
                        sy, sx = taps[ki]
                        tcl = phc.tile([128, RT, 260], BF16, tag=f"ts{ki % 4}")
                        nc.vector.tensor_tensor(
                            out=tcl, in0=wtile(ki, sx),
                            in1=xh_sb[:, r0 + 2 + sy:r0 + 2 + RT + sy, 0:260],
                            op=AL.mult)
                        for j in range(nch):
                            nc.tensor.matmul(
                                acc[:, j * 512:(j + 1) * 512], id_sb,
                                tcl[:, 2 * j:2 * j + 2, 2 + sx:258 + sx],
                                start=(ki == 0), stop=False)
                    # subtract swapped-half center via permuted negative identity
                    for j in range(nch):
                        nc.tensor.matmul(acc[:, j * 512:(j + 1) * 512], nsw_sb,
                                         xh_sb[:, r0 + 2 + 2 * j:r0 + 4 + 2 * j,
                                               2:2 + W],
                                         start=False, stop=True)
                    outs = phc.tile([128, FT], FP32)
                    nc.scalar.copy(out=outs, in_=acc)
                    nc.gpsimd.dma_start(out=out_d.ap()[:, t * FT:(t + 1) * FT],
                                        in_=outs)
    nc.compile()
    return nc


def _prep_inputs(x1, x2, dw_w, dw_b, pw_w):
    bf = ml_dtypes.bfloat16
    xcat = np.concatenate([x1, x2], axis=1)  # [N,128,H,W] f32
    xpad = np.pad(xcat, ((0, 0), (0, 0), (2, 2), (2, 2))).astype(bf)
    wb = np.concatenate([dw_w.reshape(128, 25), dw_b.reshape(128, 1)],
                        axis=1).astype(np.float32)
    pwm = (pw_w.reshape(4, 128).T * DSCALE).astype(bf)  # [128,4]
    dgm = np.zeros((len(PE_TAPS) * 128, 128), dtype=bf)
    for ki, k in enumerate(PE_TAPS):
        dy, dx = divmod(k, 5)
        np.fill_diagonal(dgm[ki * 128:(ki + 1) * 128], dw_w[:, 0, dy, dx].astype(bf))
    idm = np.eye(128, dtype=bf)
    nswm = np.zeros((128, 128), dtype=bf)
    for m in range(128):
        nswm[(m + 64) % 128, m] = -1.0
    in_maps = []
    for c in range(8):
        n, s = c // 2, c % 2
        in_maps.append({
            "xh": np.ascontiguousarray(xpad[n, :, 128 * s:128 * s + 132, :]),
            "wb": wb, "pw": pwm, "dg": dgm, "ident": idm, "nsw": nswm,
        })
    return in_maps


def _run(x1, x2, dw_w, dw_b, pw_w, trace=False):
    if "nc" not in _CACHE:
        _CACHE["nc"] = _build()
    in_maps = _prep_inputs(np.asarray(x1, np.float32), np.asarray(x2, np.float32),
                           np.asarray(dw_w, np.float32), np.asarray(dw_b, np.float32),
                           np.asarray(pw_w, np.float32))
    res = run_bass_kernel_spmd(_CACHE["nc"], in_maps, core_ids=list(range(8)),
                               trace=trace)
    o1 = np.empty((N, C, H, W), np.float32)
    o2 = np.empty((N, C, H, W), np.float32)
    for c in range(8):
        n, s = c // 2, c % 2
        o = res.results[c]["out"].reshape(128, HH, W)
        o1[n, :, 128 * s:128 * (s + 1), :] = o[:64]
        o2[n, :, 128 * s:128 * (s + 1), :] = o[64:]
    return (o1, o2), res


def kernel(x1, x2, dw_w, dw_b, pw_w):
    (o1, o2), _ = _run(x1, x2, dw_w, dw_b, pw_w, trace=False)
    return (o1, o2)



# revision 4
# speedup vs baseline: 1.3198x; 1.3198x over previous
"""Trainium2 Bass kernel for nn_FDAF (concat -> depthwise5x5 -> InstanceNorm ->
GELU -> 1x1 conv -> bilinear warp -> subtract), data-parallel over 8 cores.

Sharding: core c = (sample n = c//2, row-half s = c%2). Each core computes both
feature outputs for its 128-row half of its sample. InstanceNorm stats are
combined across the row-half pair with a tiny AllReduce collective.

v3: depthwise conv runs entirely on the PE as fp8e4m3 DoubleRow diag-pair
matmuls (2 taps per pass at 0.5 cyc/col); y stored fp8; output stored bf16;
warp products split DVE/GPSIMD with sx-aligned pair-adds so 9 taps merge in
6 PE passes. The conv bias is dropped: InstanceNorm cancels any per-channel
constant exactly.

Self-contained: hardcodes N=4, C=64, H=W=256.
"""
import numpy as np
import ml_dtypes

import concourse.bass as bass
import concourse.bacc as bacc
import concourse.tile as tile
from concourse import mybir
from concourse.bass_utils import run_bass_kernel_spmd

FP32 = mybir.dt.float32
BF16 = mybir.dt.bfloat16
FP8 = mybir.dt.float8e4
AL = mybir.AluOpType
AF = mybir.ActivationFunctionType
AX = mybir.AxisListType
MM = mybir.MatmulPerfMode

N, C, H, W = 4, 64, 256, 256
HH = 128          # rows per core (half image)
RT = 8            # rows per tile
NT = HH // RT     # 16 tiles
FT = RT * W       # free elems per tile = 2048
FH = HH * W       # free elems per half = 32768
DSCALE = (W - 1) / (2.0 * W)  # flow -> pixel displacement (align_corners=True)

# conv taps paired for DoubleRow passes (2 taps per PE pass; last pad zero-wt)
PAIRS = [(2 * i, 2 * i + 1) for i in range(12)] + [(24, 24)]

_CACHE = {}


def _build(timing=False):
    nc = bacc.Bacc("TRN2", target_bir_lowering=False, debug=False,
                   num_devices=1 if timing else 8)

    xh = nc.dram_tensor("xh", [128, 132, 260], BF16, kind="ExternalInput")
    x8 = nc.dram_tensor("x8", [128, 132, 260], FP8, kind="ExternalInput")
    dgp = nc.dram_tensor("dgp", [128, 13 * 256], FP8, kind="ExternalInput")
    pw = nc.dram_tensor("pw", [128, 4], BF16, kind="ExternalInput")
    ident = nc.dram_tensor("ident", [128, 128], BF16, kind="ExternalInput")
    nsw = nc.dram_tensor("nsw", [128, 128], BF16, kind="ExternalInput")
    out_d = nc.dram_tensor("out", [128, FH], BF16, kind="ExternalOutput")

    cc_in = nc.dram_tensor("cc_in", [128, 2], FP32, kind="Internal")
    cc_out = nc.dram_tensor("cc_out", [128, 2], FP32, kind="Internal")
    FH2 = FH // 2
    flow_hs = [nc.dram_tensor(f"flow_d{h}", [4, FH2], BF16, kind="Internal")
               for h in range(2)]
    WL = 64 * 260 + 8   # padded-row weight map: 260-stride rows + slack
    w9_hs = [nc.dram_tensor(f"w9_d{h}", [18, WL], BF16, kind="Internal")
             for h in range(2)]

    with tile.TileContext(nc) as tc:
        with tc.tile_pool(name="singles", bufs=1) as singles:
            xh_sb = singles.tile([128, 132, 260], BF16)
            for ci in range(11):
                r0c, r1c = ci * 12, min(132, ci * 12 + 12)
                eng = (nc.sync, nc.scalar)[ci % 2]
                eng.dma_start(out=xh_sb[:, r0c:r1c, :],
                              in_=xh.ap()[:, r0c:r1c, :])
            pw_sb = singles.tile([128, 4], BF16)
            nc.sync.dma_start(out=pw_sb, in_=pw.ap())
            id_sb = singles.tile([128, 128], BF16)
            nc.sync.dma_start(out=id_sb, in_=ident.ap())
            nsw_sb = singles.tile([128, 128], BF16)
            nc.sync.dma_start(out=nsw_sb, in_=nsw.ap())
            eps_t = singles.tile([128, 1], FP32)
            nc.vector.memset(eps_t, 1e-5)
            sp = singles.tile([128, NT], FP32)
            s2p = singles.tile([128, NT], FP32)
            stat = singles.tile([128, 8], FP32)

            # ---------------- Phase A: fp8 DoubleRow conv + stats -------------
            with tc.tile_pool(name="ab", bufs=1) as ab:
              x8_sb = ab.tile([128, 132, 260], FP8)
              for ci in range(6):
                  r0c, r1c = ci * 22, min(132, ci * 22 + 22)
                  eng = (nc.sync, nc.scalar)[ci % 2]
                  eng.dma_start(out=x8_sb[:, r0c:r1c, :],
                                in_=x8.ap()[:, r0c:r1c, :])
              dgp_sb = ab.tile([128, 13, 2, 128], FP8)
              nc.sync.dma_start(
                  out=dgp_sb, in_=dgp.ap().rearrange("p (i s m) -> p i s m",
                                                     i=13, s=2))
              y8 = ab.tile([128, FH], FP8)
              with tc.tile_pool(name="psA", bufs=2, space="PSUM") as psA:
                for t in range(NT):
                    r0 = t * RT
                    ps = psA.tile([128, FT], FP32)
                    for i, (ka, kb) in enumerate(PAIRS):
                        dya, dxa = divmod(ka, 5)
                        dyb, dxb = divmod(kb, 5)
                        for j in range(RT // 2):
                            ra = r0 + dya + 2 * j
                            rb = r0 + dyb + 2 * j
                            sa = x8_sb[:, ra:ra + 2, dxa:dxa + W]
                            sb_ = x8_sb[:, rb:rb + 2, dxb:dxb + W]
                            src = bass.AP(
                                tensor=sa.tensor, offset=sa.offset,
                                ap=[list(sa.ap[0]),
                                    [sb_.offset - sa.offset, 2],
                                    list(sa.ap[1]), list(sa.ap[2])])
                            nc.tensor.matmul(
                                ps[:, j * 512:(j + 1) * 512], dgp_sb[:, i],
                                src, start=(i == 0), stop=(i == 12),
                                perf_mode=MM.DoubleRow)

                    # stats from psum + store fp8 (Copy pass writes Y;
                    # Square scratch writes the same region first, WAW-ordered)
                    nc.scalar.activation(out=y8[:, t * FT:(t + 1) * FT], in_=ps,
                                         func=AF.Square,
                                         accum_out=s2p[:, t:t + 1])
                    nc.scalar.activation(out=y8[:, t * FT:(t + 1) * FT], in_=ps,
                                         func=AF.Copy, accum_out=sp[:, t:t + 1])

              # ------------- stats combine (collective over the pair) -----------
              nc.vector.tensor_reduce(out=stat[:, 0:1], in_=sp, axis=AX.X, op=AL.add)
              nc.vector.tensor_reduce(out=stat[:, 1:2], in_=s2p, axis=AX.X, op=AL.add)
              nc.sync.dma_start(out=cc_in.ap(), in_=stat[:, 0:2])
              if not timing:
                  nc.gpsimd.collective_compute(
                      kind="AllReduce", op=AL.add,
                      replica_groups=[[0, 1], [2, 3], [4, 5], [6, 7]],
                      ins=[cc_in.ap()], outs=[cc_out.ap()])
              else:
                  nc.sync.dma_start(out=cc_out.ap(), in_=cc_in.ap())
              nc.sync.dma_start(out=stat[:, 0:2], in_=cc_out.ap())
              inv = 1.0 / (H * W)
              nc.vector.tensor_scalar(out=stat[:, 2:3], in0=stat[:, 0:1],
                                      scalar1=inv, scalar2=None, op0=AL.mult)
              nc.vector.tensor_scalar(out=stat[:, 3:4], in0=stat[:, 1:2],
                                      scalar1=inv, scalar2=None, op0=AL.mult)
              nc.vector.scalar_tensor_tensor(
                  out=stat[:, 4:5], in0=stat[:, 2:3], scalar=stat[:, 2:3],
                  in1=stat[:, 3:4], op0=AL.mult, op1=AL.subtract)
              nc.scalar.activation(out=stat[:, 5:6], in_=stat[:, 4:5],
                                   func=AF.Sqrt, scale=-1.0, bias=eps_t[:, 0:1])
              nc.vector.reciprocal(out=stat[:, 6:7], in_=stat[:, 5:6])
              nc.vector.tensor_scalar(out=stat[:, 7:8], in0=stat[:, 2:3],
                                      scalar1=stat[:, 6:7], scalar2=-1.0,
                                      op0=AL.mult, op1=AL.mult)

              # --------------- Phase B: gelu + 1x1 conv -> flow_d ---------------
              with tc.tile_pool(name="phb", bufs=2) as phb, \
                   tc.tile_pool(name="psB", bufs=2, space="PSUM") as psB:
                  for t in range(NT):
                      h, tl = t // (NT // 2), t % (NT // 2)
                      g = phb.tile([128, FT], BF16)
                      nc.scalar.activation(out=g, in_=y8[:, t * FT:(t + 1) * FT],
                                           func=AF.Gelu, scale=stat[:, 6:7],
                                           bias=stat[:, 7:8])
                      psf = psB.tile([4, FT], FP32)
                      for j in range(FT // 512):
                          nc.tensor.matmul(psf[:, j * 512:(j + 1) * 512], pw_sb,
                                           g[:, j * 512:(j + 1) * 512],
                                           start=True, stop=True)
                      fls = phb.tile([4, FT], BF16)
                      nc.scalar.copy(out=fls, in_=psf)
                      nc.sync.dma_start(
                          out=flow_hs[h].ap()[:, tl * FT:(tl + 1) * FT], in_=fls)

            # -------- compact weight maps: deltas -> 18 tap products ----------
            # per half: cx/cy [64, FH2/32]; part 0-31 field1, 32-63 field2
            with tc.tile_pool(name="cw", bufs=2) as cw:
              for h in range(2):
                FC = FH2 // 32
                cx = cw.tile([64, FC], BF16, tag="cx")
                cy = cw.tile([64, FC], BF16, tag="cy")
                for (dst, r1, r2) in ((cx, 0, 2), (cy, 1, 3)):
                    for (p0, row) in ((0, r1), (32, r2)):
                        nc.sync.dma_start(
                            out=dst[p0:p0 + 32, :],
                            in_=flow_hs[h].ap()[row:row + 1, :].rearrange(
                                "a (p f) -> (a p) f", p=32))
                wsel = {}
                for ax, d in (("x", cx), ("y", cy)):
                    wp = cw.tile([64, FC], BF16, tag=f"wp{ax}")
                    wm = cw.tile([64, FC], BF16, tag=f"wm{ax}")
                    w0 = cw.tile([64, FC], BF16, tag=f"w0{ax}")
                    nc.vector.tensor_scalar(out=wp, in0=d, scalar1=0.0,
                                            scalar2=None, op0=AL.max)
                    nc.vector.tensor_scalar(out=wm, in0=d, scalar1=-1.0,
                                            scalar2=0.0, op0=AL.mult, op1=AL.max)
                    nc.scalar.activation(out=w0, in_=d, func=AF.Abs)
                    nc.vector.tensor_scalar(out=w0, in0=w0, scalar1=-1.0,
                                            scalar2=1.0, op0=AL.mult, op1=AL.add)
                    wsel[ax] = {-1: wm, 0: w0, 1: wp}
                for ki, (sy, sx) in enumerate(
                        (sy, sx) for sy in (-1, 0, 1) for sx in (-1, 0, 1)):
                    p9 = cw.tile([64, FC], BF16, tag="p9")
                    nc.vector.tensor_tensor(out=p9, in0=wsel["y"][sy],
                                            in1=wsel["x"][sx], op=AL.mult)
                    wd = w9_hs[h].ap()
                    for f in range(2):
                        dst = bass.AP(tensor=wd.tensor,
                                      offset=(ki + 9 * f) * WL + 6,
                                      ap=[[520, 32], [260, 2], [1, 256]])
                        nc.sync.dma_start(out=dst, in_=p9[32 * f:32 * f + 32, :])

            # ---------------- Phase C: warp + subtract ----------------
            # tap ki = 3*(sy+1)+(sx+1); per sx-group one DVE pair + one single.
            # Products: DVE x7, GPSIMD x2 (ki 4, 7). Pair-adds align sx so each
            # merged tile needs one PE pass: 6 merge tiles + nsw subtract.
            with tc.tile_pool(name="phc", bufs=2) as phc, \
                 tc.tile_pool(name="psC", bufs=2, space="PSUM") as psC:
                taps = [(sy, sx) for sy in (-1, 0, 1) for sx in (-1, 0, 1)]
                WT_ENG = {0: nc.sync, 1: nc.sync, 2: nc.sync, 3: nc.scalar,
                          5: nc.scalar, 6: nc.scalar,
                          4: nc.gpsimd, 7: nc.gpsimd, 8: nc.gpsimd}
                POOL_PROD = (4, 7)
                # merge groups: (members, merge_sx)
                GROUPS = [((0, 3), -1), ((1, 7), 0), ((2, 5), 1),
                          ((6,), -1), ((4,), 0), ((8,), 1)]
                for t in range(NT):
                    h, tl = t // (NT // 2), t % (NT // 2)
                    w9 = w9_hs[h].ap()
                    r0 = t * RT
                    acc = psC.tile([128, FT], FP32)
                    nch = FT // 512
                    RW = RT * 260

                    prods = {}
                    for ki in range(9):
                        sy, sx = taps[ki]
                        # pre-shifted by -sx: wt[., r, c] = w9[y=tl*8+r, c-2-sx]
                        wt = phc.tile([128, RT, 260], BF16, tag=f"w9t{ki % 3}",
                                      bufs=3)
                        src = bass.AP(tensor=w9.tensor,
                                      offset=ki * WL + 4 - sx + tl * RW,
                                      ap=[[9 * WL, 2], [0, 64], [1, RW]])
                        WT_ENG[ki].dma_start(out=wt, in_=src)
                        tcl = phc.tile([128, RT, 260], BF16, tag=f"ts{ki % 3}",
                                       bufs=3)
                        eng = nc.gpsimd if ki in POOL_PROD else nc.vector
                        eng.tensor_tensor(
                            out=tcl, in0=wt,
                            in1=xh_sb[:, r0 + 2 + sy:r0 + 2 + RT + sy, 0:260],
                            op=AL.mult)
                        prods[ki] = tcl

                    merged = []
                    for members, msx in GROUPS:
                        if len(members) == 2:
                            ka, kb = members
                            nc.vector.tensor_tensor(
                                out=prods[ka], in0=prods[ka], in1=prods[kb],
                                op=AL.add)
                            merged.append((prods[ka], msx))
                        else:
                            merged.append((prods[members[0]], msx))
                    for m, msx in merged:
                        for j in range(nch):
                            nc.tensor.matmul(
                                acc[:, j * 512:(j + 1) * 512], id_sb,
                                m[:, 2 * j:2 * j + 2, 2 + msx:258 + msx],
                                start=(m is merged[0][0]), stop=False)
                    # subtract swapped-half center via permuted negative identity
                    for j in range(nch):
                        nc.tensor.matmul(acc[:, j * 512:(j + 1) * 512], nsw_sb,
                                         xh_sb[:, r0 + 2 + 2 * j:r0 + 4 + 2 * j,
                                               2:2 + W],
                                         start=False, stop=True)
                    outs = phc.tile([128, FT], BF16)
                    nc.scalar.copy(out=outs, in_=acc)
                    nc.scalar.dma_start(out=out_d.ap()[:, t * FT:(t + 1) * FT],
                                        in_=outs)
    nc.compile()
    return nc


def _prep_inputs(x1, x2, dw_w, dw_b, pw_w):
    bf = ml_dtypes.bfloat16
    f8 = ml_dtypes.float8_e4m3
    xcat = np.concatenate([x1, x2], axis=1)  # [N,128,H,W] f32
    xpad = np.pad(xcat, ((0, 0), (0, 0), (2, 2), (2, 2)))
    xpad_bf = xpad.astype(bf)
    xpad_f8 = xpad.astype(f8)
    w25 = dw_w.reshape(128, 25).astype(f8)
    dgpm = np.zeros((128, 13, 2, 128), dtype=f8)
    rr = np.arange(128)
    for i, (ka, kb) in enumerate(PAIRS):
        dgpm[rr, i, 0, rr] = w25[:, ka]
        if i < 12:
            dgpm[rr, i, 1, rr] = w25[:, kb]
        # last pair: second slice stays zero (pad tap)
    pwm = (pw_w.reshape(4, 128).T * DSCALE).astype(bf)  # [128,4]
    idm = np.eye(128, dtype=bf)
    nswm = np.zeros((128, 128), dtype=bf)
    for m in range(128):
        nswm[(m + 64) % 128, m] = -1.0
    in_maps = []
    for c in range(8):
        n, s = c // 2, c % 2
        in_maps.append({
            "xh": np.ascontiguousarray(xpad_bf[n, :, 128 * s:128 * s + 132, :]),
            "x8": np.ascontiguousarray(xpad_f8[n, :, 128 * s:128 * s + 132, :]),
            "dgp": np.ascontiguousarray(dgpm.reshape(128, 13 * 256)),
            "pw": pwm, "ident": idm, "nsw": nswm,
        })
    return in_maps


def _run(x1, x2, dw_w, dw_b, pw_w, trace=False):
    if "nc" not in _CACHE:
        _CACHE["nc"] = _build()
    in_maps = _prep_inputs(np.asarray(x1, np.float32), np.asarray(x2, np.float32),
                           np.asarray(dw_w, np.float32), np.asarray(dw_b, np.float32),
                           np.asarray(pw_w, np.float32))
    res = run_bass_kernel_spmd(_CACHE["nc"], in_maps, core_ids=list(range(8)),
                               trace=trace)
    o1 = np.empty((N, C, H, W), np.float32)
    o2 = np.empty((N, C, H, W), np.float32)
    for c in range(8):
        n, s = c // 2, c % 2
        o = res.results[c]["out"].astype(np.float32).reshape(128, HH, W)
        o1[n, :, 128 * s:128 * (s + 1), :] = o[:64]
        o2[n, :, 128 * s:128 * (s + 1), :] = o[64:]
    return (o1, o2), res


def kernel(x1, x2, dw_w, dw_b, pw_w):
    (o1, o2), _ = _run(x1, x2, dw_w, dw_b, pw_w, trace=False)
    return (o1, o2)
